# revision 16
# baseline (speedup 1.0000x reference)
"""Trainium2 Bass kernel: complex-valued transformer block (nn_EqModelComplex).

v2 design:
- 8 cores = (batch b in 0..3) x (query-parity hh in 0..1). Each core owns the
  512 queries with global index == hh (mod 2) of one batch element. Host
  permutes the token axis per core to [own queries | other parity], so the
  causal structure of every attention tile is IDENTICAL on all cores
  (triangular diagonal blocks), and the LN1 output for chunk 0 doubles as the
  Q-path input. No collectives.
- All weight preprocessing (exp/cos/sin of the magnitude-phase params) is done
  on the host in numpy; the device receives packed bf16 weight components.
  Q/K/O/gate/up use the 3-multiply Karatsuba complex form
  (m1=wr@(xr+xi), m2=(wr+wi)@xi, m3=(wi-wr)@xr; yr=m1-m2, yi=m1+m3);
  V (tokens-stationary) and down use the plain 4-multiply form with a host
  negated component.
- Feature-major layout: SBUF tiles are (features on partitions, tokens free).
  All matmuls in bf16 (x arrives bf16 from the host). LN stats via bf16
  ones-matmuls; the per-token stat / softmax-denominator broadcasts use gpsimd
  partition_broadcast (Pool engine). LN stats chains are split so a chunk's
  reduction hides under the previous chunk's projections.
- RoPE: rotate-half done on the PE as a constant signed-permutation matmul
  (shmat), leaving 3 bf16 DVE ops per rotation.
- Attention: transposed scores (tk on partitions, tq free), score/exp/AV all
  column-trimmed to the causal region, exp (no max subtraction) straight to
  bf16, multiplicative diagonal-block mask, denominators via a ones-column in
  V; score/exp/mask pipelined LAG tiles ahead of the AV accumulation.
- Activation-table discipline: only Sqrt and Exp/Tanh sets are used, loads are
  batched (sigmoid(x) computed as 0.5*(1+tanh(x/2))).
"""
import sys, os
sys.path.insert(0, '/opt/trn_rl_repo')
import math
import numpy as np
from contextlib import ExitStack

P = 128
D = 512
S = 1024
B = 4
H = 8
HD = 64
HID = 2048
TQ = 512
FT = D // P          # 4
NCORES = 8
EPS = 1e-6
SCALE = 1.0 / math.sqrt(HD)

_CACHE = {}
ABLATE = set()   # timing-ablation flags: 'noatt', 'noffn', 'nowdma'


def _emit_body(nc, tc, io, TRIV_LN=False, TRIV_B=False):
    from concourse import mybir

    dt = mybir.dt
    AF = mybir.ActivationFunctionType
    ALU = mybir.AluOpType
    f32 = dt.float32
    f32r = dt.float32r
    bf16 = dt.bfloat16
    TTv = nc.vector.tensor_tensor      # DVE
    TTp = nc.gpsimd.tensor_tensor      # Pool
    TSv = nc.vector.tensor_scalar
    STTv = nc.vector.scalar_tensor_tensor

    ctx = ExitStack()
    with ctx:
        # ---------------- long-lived pools ----------------
        const = ctx.enter_context(tc.tile_pool(name="const", bufs=1))
        p_tmp = ctx.enter_context(tc.tile_pool(name="p_tmp", bufs=1))
        p_sm = ctx.enter_context(tc.tile_pool(name="p_sm", bufs=1))
        p_bc = ctx.enter_context(tc.tile_pool(name="p_bc", bufs=1))

        es_ps = ExitStack()
        ps = es_ps.enter_context(tc.tile_pool(name="ps", bufs=6, space="PSUM"))

        ones_b = const.tile([P, 1], bf16)
        nc.vector.memset(ones_b, 1.0)
        shmat = const.tile([P, P], bf16, name='shmat')
        nc.sync.dma_start(out=shmat, in_=io['shmat'][:])
        ceps = const.tile([1, 1], f32)
        nc.vector.memset(ceps, EPS)

        lncols = {}
        if not TRIV_LN:
            for key in ['ln1_gr', 'ln1_gi', 'ln1_br', 'ln1_bi',
                        'ln2_gr', 'ln2_gi', 'ln2_br', 'ln2_bi']:
                c = const.tile([P, FT], f32, name='c_' + key)
                nc.sync.dma_start(out=c, in_=io[key].rearrange("(t p) -> p t", p=P))
                lncols[key] = c
        bcols = {}
        if not TRIV_B:
            for nm in ['q', 'k', 'v', 'o']:
                br = const.tile([P, FT], f32, name='cb_r_' + nm)
                bi = const.tile([P, FT], f32, name='cb_i_' + nm)
                nc.sync.dma_start(out=br, in_=io['b_' + nm + '_r'].rearrange("(t p) -> p t", p=P))
                nc.sync.dma_start(out=bi, in_=io['b_' + nm + '_i'].rearrange("(t p) -> p t", p=P))
                bcols[nm] = (br, bi)

        # x load (full permuted sequence, feature-major)
        es_x = ExitStack()
        p_x = es_x.enter_context(tc.tile_pool(name="p_x", bufs=1, side='right'))
        xf_r = [p_x.tile([P, S], bf16, name='xfr%d' % kt) for kt in range(FT)]
        xf_i = [p_x.tile([P, S], bf16, name='xfi%d' % kt) for kt in range(FT)]
        es_wq = ExitStack()
        p_wq = es_wq.enter_context(tc.tile_pool(name="p_wq", bufs=1, side='right'))
        wqkvo = [p_wq.tile([P, 4 * 3 * D], bf16, name='wqkvo%d' % kt) for kt in range(FT)]
        ck = const.tile([P, S], bf16, name='ck')
        sk = const.tile([P, S], bf16, name='sk')
        maskD = const.tile([P, 8 * P], bf16, name='maskD')

        def dma_x(ch):
            csl = slice(ch * TQ, (ch + 1) * TQ)
            for kt in range(FT):
                nc.sync.dma_start(out=xf_r[kt][:, csl], in_=io['xf_r'][kt * P:(kt + 1) * P, csl])
                nc.sync.dma_start(out=xf_i[kt][:, csl], in_=io['xf_i'][kt * P:(kt + 1) * P, csl])

        def dma_w(m):
            if 'nowdma' in ABLATE:
                return
            ms = slice(m * 3 * D, (m + 1) * 3 * D)
            for kt in range(FT):
                nc.sync.dma_start(out=wqkvo[kt][:, ms],
                                  in_=io['wqkvo'][kt * P:(kt + 1) * P, ms])

        # host-computed LN1 stats: per chunk [iv | mr*iv | mi*iv]
        lnst_t = const.tile([1, 2 * 3 * TQ], bf16, name='lnst_t')
        nc.sync.dma_start(out=lnst_t, in_=io['lnst'][:])

        # DMA order = first-consumer order: x ch0 (LN1), k weights, x ch1,
        # rope tables, then q/v/o weights
        dma_x(0)
        dma_w(1)
        dma_x(1)
        nc.sync.dma_start(out=ck, in_=io['cosk'][:])
        nc.sync.dma_start(out=sk, in_=io['sink'][:])
        dma_w(0)
        dma_w(2)
        dma_w(3)
        nc.sync.dma_start(out=maskD, in_=io['maskD'][:])

        def wsl(kt, m, c):
            # slice for matrix m (0=q,1=k,2=v,3=o) component c
            base = (m * 3 + c) * D
            return wqkvo[kt][:, base:base + D]

        # ---------- layernorm (NT tokens, feature-major), split in two ----------
        # ln_stats emits the reduction + broadcast chain; ln_normalize consumes
        # the broadcast tiles. Splitting lets a later chunk's stats chain hide
        # under the previous chunk's projections.
        # xbf=True means the x_r/x_i APs are already bf16 (host-quantized x);
        # otherwise (f32 residuals) bf16 staging copies feed the stat matmuls.
        def ln_stats(x_r, x_i, xbf=False):
            NT = x_r[0].shape[-1]
            s_ps = [ps.tile([1, NT], f32, tag="ps", name="lnps%d" % q) for q in range(3)]
            for kt in range(FT):
                if xbf:
                    xrb, xib = x_r[kt], x_i[kt]
                else:
                    xrb = p_tmp.tile([P, NT], bf16, tag="ln_xrb", bufs=2)
                    xib = p_tmp.tile([P, NT], bf16, tag="ln_xib", bufs=2)
                    nc.gpsimd.tensor_copy(out=xrb, in_=x_r[kt])
                    nc.scalar.copy(out=xib, in_=x_i[kt])
                ta = p_tmp.tile([P, NT], bf16, tag="ln_tab", bufs=2)
                tb = p_tmp.tile([P, NT], bf16, tag="ln_tbb", bufs=2)
                STTv(out=ta, in0=x_r[kt], scalar=1.0, in1=x_r[kt], op0=ALU.mult,
                     op1=ALU.mult)
                STTv(out=tb, in0=x_i[kt], scalar=1.0, in1=x_i[kt], op0=ALU.mult,
                     op1=ALU.mult)
                st, sp = kt == 0, kt == FT - 1
                nc.tensor.matmul(s_ps[0], ones_b, xrb, start=st, stop=sp)
                nc.tensor.matmul(s_ps[1], ones_b, xib, start=st, stop=sp)
                nc.tensor.matmul(s_ps[2], ones_b, ta, start=st, stop=False)
                nc.tensor.matmul(s_ps[2], ones_b, tb, start=False, stop=sp)
            # small [1,NT] ops — all on DVE (except the Sqrt, which only ACT
            # has) so the chain doesn't pay a cross-engine semaphore hop per op
            mr = p_sm.tile([1, NT], f32, tag="sm_mr", bufs=1)
            mi = p_sm.tile([1, NT], f32, tag="sm_mi", bufs=1)
            vv = p_sm.tile([1, NT], f32, tag="sm_vv", bufs=1)
            t2 = p_sm.tile([1, NT], f32, tag="sm_t2", bufs=1)
            TSv(out=mr, in0=s_ps[0], scalar1=1.0 / D, scalar2=None, op0=ALU.mult)
            TSv(out=mi, in0=s_ps[1], scalar1=1.0 / D, scalar2=None, op0=ALU.mult)
            TTv(out=t2, in0=mr, in1=mr, op=ALU.mult)
            STTv(out=vv, in0=s_ps[2], scalar=1.0 / D, in1=t2, op0=ALU.mult, op1=ALU.subtract)
            TTv(out=t2, in0=mi, in1=mi, op=ALU.mult)
            TTv(out=vv, in0=vv, in1=t2, op=ALU.subtract)
            sm3 = p_sm.tile([1, 3 * NT], bf16, tag="sm_sm3", bufs=2)
            nc.scalar.activation(out=vv, in_=vv, func=AF.Sqrt, bias=ceps)
            nc.vector.reciprocal(out=vv, in_=vv)                      # iv
            bc3 = p_bc.tile([P, 3 * NT], bf16, tag="bc_3", bufs=2)
            # iv broadcasts first: the normalize's leading multiply only needs
            # iv, so the mean broadcasts come off the critical path
            nc.vector.tensor_copy(out=sm3[:, 0:NT], in_=vv)
            nc.gpsimd.partition_broadcast(bc3[:, 0:NT], sm3[:, 0:NT])
            TTv(out=sm3[:, NT:2 * NT], in0=mr, in1=vv, op=ALU.mult)   # mr*iv
            TTv(out=sm3[:, 2 * NT:3 * NT], in0=mi, in1=vv, op=ALU.mult)
            nc.gpsimd.partition_broadcast(bc3[:, NT:3 * NT], sm3[:, NT:3 * NT])
            return bc3[:, 0:NT], bc3[:, NT:2 * NT], bc3[:, 2 * NT:3 * NT]

        def ln_normalize(bc3, x_r, x_i, ln, dst_r, dst_i, dst_s, hpool, htag, hbufs):
            NT = x_r[0].shape[-1]
            iv_bc, mr_bc, mi_bc = bc3
            gcols = None if TRIV_LN else (lncols[ln + '_gr'], lncols[ln + '_gi'],
                                          lncols[ln + '_br'], lncols[ln + '_bi'])
            for kt in range(FT):
                tr = p_tmp.tile([P, NT], bf16, tag="ln_tr", bufs=2)
                ti = p_tmp.tile([P, NT], bf16, tag="ln_ti", bufs=2)
                idx = len(dst_r)
                if hbufs == 0:
                    hr = hpool.tile([P, NT], bf16, name=htag + "hr%d" % idx, uniquify=True)
                    hi = hpool.tile([P, NT], bf16, name=htag + "hi%d" % idx, uniquify=True)
                    hs = hpool.tile([P, NT], bf16, name=htag + "hs%d" % idx,
                                    uniquify=True)
                else:
                    hr = hpool.tile([P, NT], bf16, tag=htag + "r", bufs=hbufs,
                                    name=htag + "hr", uniquify=True)
                    hi = hpool.tile([P, NT], bf16, tag=htag + "i", bufs=hbufs,
                                    name=htag + "hi", uniquify=True)
                    hs = hpool.tile([P, NT], bf16, tag=htag + "s", bufs=hbufs,
                                    name=htag + "hs", uniquify=True)
                TTv(out=tr, in0=x_r[kt], in1=iv_bc, op=ALU.mult)
                TTv(out=ti, in0=x_i[kt], in1=iv_bc, op=ALU.mult)
                if TRIV_LN:
                    TTv(out=hr, in0=tr, in1=mr_bc, op=ALU.subtract)
                    TTp(out=hi, in0=ti, in1=mi_bc, op=ALU.subtract)
                else:
                    nr = p_tmp.tile([P, NT], f32, tag="ln_nr", bufs=2)
                    ni = p_tmp.tile([P, NT], f32, tag="ln_ni", bufs=2)
                    TTp(out=nr, in0=tr, in1=mr_bc, op=ALU.subtract)
                    TTp(out=ni, in0=ti, in1=mi_bc, op=ALU.subtract)
                    grc, gic, brc, bic = gcols
                    ta = p_tmp.tile([P, NT], f32, tag="ln_ta", bufs=2)
                    tb = p_tmp.tile([P, NT], f32, tag="ln_tb", bufs=2)
                    TSv(out=ta, in0=nr, scalar1=grc[:, kt:kt + 1], scalar2=None, op0=ALU.mult)
                    TSv(out=tb, in0=ni, scalar1=gic[:, kt:kt + 1], scalar2=None, op0=ALU.mult)
                    TTv(out=ta, in0=ta, in1=tb, op=ALU.subtract)
                    TSv(out=hr, in0=ta, scalar1=brc[:, kt:kt + 1], scalar2=None, op0=ALU.add)
                    TSv(out=ta, in0=nr, scalar1=gic[:, kt:kt + 1], scalar2=None, op0=ALU.mult)
                    TSv(out=tb, in0=ni, scalar1=grc[:, kt:kt + 1], scalar2=None, op0=ALU.mult)
                    TTv(out=ta, in0=ta, in1=tb, op=ALU.add)
                    TSv(out=hi, in0=ta, scalar1=bic[:, kt:kt + 1], scalar2=None, op0=ALU.add)
                TTv(out=hs, in0=hr, in1=hi, op=ALU.add)
                dst_r.append(hr)
                dst_i.append(hi)
                dst_s.append(hs)

        # Karatsuba complex matmul accumulation (weights stationary)
        def kmm(ps3, w3, kt, nkt, rhs_r, rhs_i, rhs_s, msl):
            m1, m2, m3 = ps3
            wr, wpw, wmw = w3
            st, sp = kt == 0, kt == nkt - 1
            nc.tensor.matmul(m1, wr[:, msl], rhs_s, start=st, stop=sp)
            nc.tensor.matmul(m2, wpw[:, msl], rhs_i, start=st, stop=sp)
            nc.tensor.matmul(m3, wmw[:, msl], rhs_r, start=st, stop=sp)

        # rope: dst (bf16) <- pre*cos + rothalf(pre)*sin. rothalf is a signed
        # partition permutation, done as a PE matmul with the constant shmat;
        # the PSUM result goes through an ACT copy, leaving 3 bf16 DVE ops.
        def rope(dst, pre, cosT, sinT):
            NT = pre.shape[-1]
            shps = ps.tile([P, NT], f32, tag="ps", name="shps")
            nc.tensor.matmul(shps, shmat, pre, start=True, stop=True)
            shb = p_kq.tile([P, NT], bf16, tag="rope_shb", bufs=2)
            nc.scalar.copy(out=shb, in_=shps)
            tmp = p_kq.tile([P, NT], bf16, tag="rope_tmp", bufs=2)
            TTv(out=dst, in0=pre, in1=cosT, op=ALU.mult)
            TTv(out=tmp, in0=shb, in1=sinT, op=ALU.mult)
            TTv(out=dst, in0=dst, in1=tmp, op=ALU.add)

        # Karatsuba combine: yr = m1-m2, yi = m1+m3. TensorTensor cannot read
        # two PSUM banks, so m1 goes through SBUF via one ACT copy first.
        def kcombine(dst_r, dst_i, m3_, bias_r=None, bias_i=None):
            m1sb = p_tmp.tile([P, dst_r.shape[-1]], f32, tag="m1sb", bufs=2)
            nc.scalar.copy(out=m1sb, in_=m3_[0])
            if bias_r is None:
                TTv(out=dst_r, in0=m1sb, in1=m3_[1], op=ALU.subtract)
                TTv(out=dst_i, in0=m1sb, in1=m3_[2], op=ALU.add)
            else:
                t = p_tmp.tile([P, dst_r.shape[-1]], f32, tag="cmb_t", bufs=2)
                TTv(out=t, in0=m1sb, in1=m3_[1], op=ALU.subtract)
                TSv(out=dst_r, in0=t, scalar1=bias_r, scalar2=None, op0=ALU.add)
                TTv(out=t, in0=m1sb, in1=m3_[2], op=ALU.add)
                TSv(out=dst_i, in0=t, scalar1=bias_i, scalar2=None, op0=ALU.add)

        # ===== pools for attention-era tiles (right stack: p_am under p_kq
        # so the bulky k/q/v/h1 tiles free right after the attention loop
        # while attn tiles survive into the O projection) =====
        es_am = ExitStack()
        p_am = es_am.enter_context(tc.tile_pool(name="p_am", bufs=1, side='right'))
        attn_r = [p_am.tile([P, TQ], bf16, name='attnr%d' % ot) for ot in range(FT)]
        attn_i = [p_am.tile([P, TQ], bf16, name='attni%d' % ot) for ot in range(FT)]
        attn_s = [p_am.tile([P, TQ], bf16, name='attns%d' % ot) for ot in range(FT)]
        es_att = ExitStack()
        p_kq = es_att.enter_context(tc.tile_pool(name="p_kq", bufs=1, side='right'))
        k_rot_r = [p_kq.tile([P, S], bf16, name='krr%d' % ot) for ot in range(FT)]
        k_rot_i = [p_kq.tile([P, S], bf16, name='kri%d' % ot) for ot in range(FT)]
        q_rot_r = [p_kq.tile([P, TQ], bf16, name='qrr%d' % ot) for ot in range(FT)]
        q_rot_i = [p_kq.tile([P, TQ], bf16, name='qri%d' % ot) for ot in range(FT)]
        vaug = [p_kq.tile([P, H, 129], bf16, name='vaug%d' % t) for t in range(8)]
        for t in range(8):
            nc.gpsimd.memset(vaug[t][:, :, 64:65], 1.0)

        bk = bcols.get('k', (None, None)) if not TRIV_B else (None, None)
        bq = bcols.get('q', (None, None)) if not TRIV_B else (None, None)

        def kq_proj(m, ot, hoff, msl):
            m3_ = [ps.tile([P, TQ], f32, tag="ps", name="kqps%d" % q) for q in range(3)]
            for kt in range(FT):
                kmm(m3_, (wsl(kt, m, 0), wsl(kt, m, 1), wsl(kt, m, 2)), kt, FT,
                    h1_r[hoff + kt], h1_i[hoff + kt], h1_s[hoff + kt], msl)
            pre_r = p_kq.tile([P, TQ], bf16, tag="pre_r", bufs=2)
            pre_i = p_kq.tile([P, TQ], bf16, tag="pre_i", bufs=2)
            bb = bq if m == 0 else bk
            bcr = None if TRIV_B else bb[0][:, ot:ot + 1]
            bci = None if TRIV_B else bb[1][:, ot:ot + 1]
            kcombine(pre_r, pre_i, m3_, bcr, bci)
            return pre_r, pre_i

        # ===== LN1 ch0 -> [ch1 stats] -> K-ch0 + Q -> ch1 normalize -> K-ch1 =====
        # (chunk 1's stats/broadcast chain hides under the chunk-0 projections)
        h1_r, h1_i, h1_s = [], [], []
        xs = []
        for ch in range(2):
            csl = slice(ch * TQ, (ch + 1) * TQ)
            xs.append(([xf_r[kt][:, csl] for kt in range(FT)],
                       [xf_i[kt][:, csl] for kt in range(FT)]))
        def ln1_bc(ch):
            bc3 = p_bc.tile([P, 3 * TQ], bf16, tag="bc_3", bufs=2)
            nc.gpsimd.partition_broadcast(bc3, lnst_t[:, ch * 1536:(ch + 1) * 1536])
            return bc3[:, 0:TQ], bc3[:, TQ:2 * TQ], bc3[:, 2 * TQ:3 * TQ]

        NOATT = 'noatt' in ABLATE
        bc0 = ln1_bc(0)
        if not NOATT:
            ln_normalize(bc0, xs[0][0], xs[0][1], 'ln1', h1_r, h1_i, h1_s, p_kq, "h1", 0)
        bc1 = ln1_bc(1)
        for ch in ([] if NOATT else range(2)):
            csl = slice(ch * TQ, (ch + 1) * TQ)
            if ch == 1:
                ln_normalize(bc1, xs[1][0], xs[1][1], 'ln1', h1_r, h1_i, h1_s,
                             p_kq, "h1", 0)
            for ot in range(FT):
                msl = slice(ot * P, (ot + 1) * P)
                pre_r, pre_i = kq_proj(1, ot, ch * FT, msl)
                rope(k_rot_r[ot][:, csl], pre_r, ck[:, csl], sk[:, csl])
                rope(k_rot_i[ot][:, csl], pre_i, ck[:, csl], sk[:, csl])
                if ch == 0:
                    pre_r, pre_i = kq_proj(0, ot, 0, msl)
                    rope(q_rot_r[ot], pre_r, ck[:, 0:TQ], sk[:, 0:TQ])
                    rope(q_rot_i[ot], pre_i, ck[:, 0:TQ], sk[:, 0:TQ])

        # ================= V proj (tokens stationary, 4-mult) =================
        for t in ([] if NOATT else range(8)):
            ch, t4 = t // 4, t % 4
            tsl = slice(t4 * P, (t4 + 1) * P)
            pvr = ps.tile([P, D], f32, tag="ps", name="pvr")
            pvi = ps.tile([P, D], f32, tag="ps", name="pvi")
            for kt in range(FT):
                st, sp = kt == 0, kt == FT - 1
                hr_t, hi_t = h1_r[ch * FT + kt], h1_i[ch * FT + kt]
                nc.tensor.matmul(pvr, hr_t[:, tsl], wsl(kt, 2, 0), start=st, stop=False)
                nc.tensor.matmul(pvr, hi_t[:, tsl], wsl(kt, 2, 2), start=False, stop=sp)
                nc.tensor.matmul(pvi, hr_t[:, tsl], wsl(kt, 2, 1), start=st, stop=False)
                nc.tensor.matmul(pvi, hi_t[:, tsl], wsl(kt, 2, 0), start=False, stop=sp)
            nc.scalar.copy(out=vaug[t][:, :, 0:64], in_=pvr.rearrange("p (h d) -> p h d", h=H))
            nc.vector.tensor_copy(out=vaug[t][:, :, 65:129],
                                  in_=pvi.rearrange("p (h d) -> p h d", h=H))

        # ================= attention =================
        es_at = ExitStack()
        ps_ar = es_at.enter_context(tc.tile_pool(name="ps_ar", bufs=1, space="PSUM"))
        ps_ai = es_at.enter_context(tc.tile_pool(name="ps_ai", bufs=1, space="PSUM"))
        bv = bcols.get('v', (None, None)) if not TRIV_B else (None, None)
        LAG = 5
        if NOATT:
            for ot in range(FT):
                nc.vector.memset(attn_r[ot], 0.01)
                nc.vector.memset(attn_i[ot], 0.01)
                nc.vector.memset(attn_s[ot], 0.02)
        for h in ([] if NOATT else range(H)):
            ot, prow = h // 2, 64 * (h % 2)
            po_r = ps_ar.tile([65, TQ], f32, tag="at_r")
            po_i = ps_ai.tile([64, TQ], f32, tag="at_i")
            p32s = {}
            # software pipeline: emit score/exp/mask LAG tiles ahead of the AV
            # accumulation so the PE never stalls on the exp/mask latency
            for step in range(8 + LAG):
                if step < 8:
                    t = step
                    c0 = 128 * (t % 4)
                    tsl = slice(t * P, (t + 1) * P)
                    csl = slice(c0, TQ)
                    pst = ps.tile([P, TQ], f32, tag="ps", name="pst")
                    nc.tensor.matmul(pst[:, csl], k_rot_r[ot][prow:prow + 64, tsl],
                                     q_rot_r[ot][prow:prow + 64, csl], start=True, stop=False)
                    nc.tensor.matmul(pst[:, csl], k_rot_i[ot][prow:prow + 64, tsl],
                                     q_rot_i[ot][prow:prow + 64, csl], start=False, stop=True)
                    p32 = p_am.tile([P, TQ], bf16, tag="p32", bufs=LAG + 1)
                    nc.scalar.activation(out=p32[:, csl], in_=pst[:, csl], func=AF.Exp,
                                         scale=SCALE)
                    TTp(out=p32[:, c0:c0 + 128], in0=p32[:, c0:c0 + 128],
                        in1=maskD[:, t * P:(t + 1) * P], op=ALU.mult)
                    p32s[t] = p32
                if step >= LAG:
                    t = step - LAG
                    c0 = 128 * (t % 4)
                    csl = slice(c0, TQ)
                    p32 = p32s.pop(t)
                    # accumulate only the causal column range; columns < c0 get
                    # no contribution from this tile (masked to zero anyway)
                    nc.tensor.matmul(po_r[:, csl], vaug[t][:, h, 0:65], p32[:, csl],
                                     start=(t == 0), stop=(t == 7), skip_group_check=True)
                    nc.tensor.matmul(po_i[:, csl], vaug[t][:, h, 65:129], p32[:, csl],
                                     start=(t == 0), stop=(t == 7), skip_group_check=True)
            rec = p_sm.tile([1, TQ], f32, tag="sm_rec", bufs=1)
            nc.vector.reciprocal(out=rec, in_=po_r[64:65, :])
            rec_bc = p_bc.tile([64, TQ], f32, tag="bc_rec", bufs=1)
            nc.gpsimd.partition_broadcast(rec_bc, rec)
            ar_sl = attn_r[ot][prow:prow + 64, :]
            ai_sl = attn_i[ot][prow:prow + 64, :]
            TTv(out=ar_sl, in0=po_r[0:64, :], in1=rec_bc, op=ALU.mult)
            TTv(out=ai_sl, in0=po_i[0:64, :], in1=rec_bc, op=ALU.mult)
            if not TRIV_B:
                TSv(out=ar_sl, in0=ar_sl, scalar1=bv[0][prow:prow + 64, ot:ot + 1],
                    scalar2=None, op0=ALU.add)
                TSv(out=ai_sl, in0=ai_sl, scalar1=bv[1][prow:prow + 64, ot:ot + 1],
                    scalar2=None, op0=ALU.add)
            if h % 2 == 1:   # both heads of this ot tile done
                TTv(out=attn_s[ot], in0=attn_r[ot], in1=attn_i[ot], op=ALU.add)
        es_at.close()
        es_att.close()   # k_rot/q_rot/vaug dead after the attention loop

        # FFN gate/up weight pool: group 0 streams in during late attention,
        # group 1 during the O projection + LN2 window.
        es_f = ExitStack()
        p_fw = es_f.enter_context(tc.tile_pool(name="p_fw", bufs=1))
        wgu_t = {}

        def load_wgu(g):
            tl = []
            for kt in range(FT):
                w = p_fw.tile([P, 2 * 3 * TQ], bf16, tag="wgu", bufs=4, name="wgu", uniquify=True)
                if 'nowdma' not in ABLATE:
                    nc.sync.dma_start(out=w, in_=io['wgu'][kt * P:(kt + 1) * P,
                                                           g * 3072:(g + 1) * 3072])
                tl.append(w)
            wgu_t[g] = tl

        NOFFN = 'noffn' in ABLATE
        if not NOFFN:
            load_wgu(0)

        # ================= O proj + residual =================
        es_keep = ExitStack()
        p_keep = es_keep.enter_context(tc.tile_pool(name="p_keep", bufs=1))
        res_r = [p_keep.tile([P, TQ], f32, name='resr%d' % ot) for ot in range(FT)]
        res_i = [p_keep.tile([P, TQ], f32, name='resi%d' % ot) for ot in range(FT)]
        resb_r = [p_keep.tile([P, TQ], bf16, name='resbr%d' % ot) for ot in range(FT)]
        resb_i = [p_keep.tile([P, TQ], bf16, name='resbi%d' % ot) for ot in range(FT)]

        if not NOFFN:
            load_wgu(1)
        xq_r = [xf_r[ot][:, 0:TQ] for ot in range(FT)]
        xq_i = [xf_i[ot][:, 0:TQ] for ot in range(FT)]
        bo = bcols.get('o', (None, None)) if not TRIV_B else (None, None)
        for ot in range(FT):
            msl = slice(ot * P, (ot + 1) * P)
            m3_ = [ps.tile([P, TQ], f32, tag="ps", name="ops%d" % q) for q in range(3)]
            for kt in range(FT):
                kmm(m3_, (wsl(kt, 3, 0), wsl(kt, 3, 1), wsl(kt, 3, 2)), kt, FT,
                    attn_r[kt], attn_i[kt], attn_s[kt], msl)
            tr = p_keep.tile([P, TQ], f32, tag="ores", bufs=2)
            ti = p_keep.tile([P, TQ], f32, tag="ores", bufs=2)
            bcr = None if TRIV_B else bo[0][:, ot:ot + 1]
            bci = None if TRIV_B else bo[1][:, ot:ot + 1]
            kcombine(tr, ti, m3_, bcr, bci)
            TTp(out=res_r[ot], in0=tr, in1=xq_r[ot], op=ALU.add)
            TTp(out=res_i[ot], in0=ti, in1=xq_i[ot], op=ALU.add)
            TTv(out=resb_r[ot], in0=tr, in1=xq_r[ot], op=ALU.add)
            TTv(out=resb_i[ot], in0=ti, in1=xq_i[ot], op=ALU.add)
        es_am.close()
        es_wq.close()
        es_x.close()   # xf no longer needed after the residual
        es_fh = ExitStack()
        p_fh = es_fh.enter_context(tc.tile_pool(name="p_fh", bufs=1))

        # ================= LN2 =================
        h2_r, h2_i, h2_s = [], [], []
        if not NOFFN:
            bc2 = ln_stats(resb_r, resb_i, xbf=True)
            ln_normalize(bc2, resb_r, resb_i, 'ln2', h2_r, h2_i, h2_s, p_keep, "h2", 0)


        # ================= FFN gate/up -> hid =================
        # hid is computed in two stages so that ALL 16 Sqrt ops and ALL 16
        # Tanh ops form contiguous ACT batches (sqrt is the only function
        # outside the exp/tanh/square/copy table set, so each sqrt<->tanh
        # alternation in the final ACT order costs a ~1.3-2.7us table load):
        #   stage 1 (per group): gate/up matmuls+combines, qq=|g|^2, and the
        #     UNGATED complex product hidp = g*u  (g,u short-lived)
        #   stage 2: batched sqrt(qq), batched tanh, then hid = s*hidp in place
        hid_r = [p_fh.tile([P, TQ], bf16, name='hidr%d' % j) for j in range(16)]
        hid_i = [p_fh.tile([P, TQ], bf16, name='hidi%d' % j) for j in range(16)]
        qs_ = [p_fh.tile([P, TQ], bf16, name='qq%d' % j) for j in range(16)]
        for g in ([] if NOFFN else range(4)):
            if g + 2 < 4:
                load_wgu(g + 2)
            gtiles = wgu_t.pop(g)
            for jj in range(4):
                j = g * 4 + jj
                jsl = slice(jj * P, (jj + 1) * P)
                mg = [ps.tile([P, TQ], f32, tag="ps", name="gps%d" % q) for q in range(3)]
                for kt in range(FT):
                    w3 = (gtiles[kt][:, 0:TQ], gtiles[kt][:, TQ:2 * TQ],
                          gtiles[kt][:, 2 * TQ:3 * TQ])
                    kmm(mg, w3, kt, FT, h2_r[kt], h2_i[kt], h2_s[kt], jsl)
                gr = p_fh.tile([P, TQ], bf16, tag="f_gr", bufs=2)
                gi = p_fh.tile([P, TQ], bf16, tag="f_gi", bufs=2)
                kcombine(gr, gi, mg)
                mu = [ps.tile([P, TQ], f32, tag="ps", name="ups%d" % q) for q in range(3)]
                for kt in range(FT):
                    w3 = (gtiles[kt][:, 3 * TQ:4 * TQ], gtiles[kt][:, 4 * TQ:5 * TQ],
                          gtiles[kt][:, 5 * TQ:6 * TQ])
                    kmm(mu, w3, kt, FT, h2_r[kt], h2_i[kt], h2_s[kt], jsl)
                ur = p_fh.tile([P, TQ], bf16, tag="f_ur", bufs=2)
                ui = p_fh.tile([P, TQ], bf16, tag="f_ui", bufs=2)
                kcombine(ur, ui, mu)
                sq1 = p_fh.tile([P, TQ], bf16, tag="f_sq1", bufs=2)
                STTv(out=sq1, in0=gr, scalar=1.0, in1=gr, op0=ALU.mult, op1=ALU.mult)
                STTv(out=qs_[j], in0=gi, scalar=1.0, in1=gi, op0=ALU.mult, op1=ALU.mult)
                TTp(out=qs_[j], in0=qs_[j], in1=sq1, op=ALU.add)
                t1 = p_fh.tile([P, TQ], bf16, tag="f_t1", bufs=2)
                t2 = p_fh.tile([P, TQ], bf16, tag="f_t2", bufs=2)
                TTv(out=t1, in0=gr, in1=ur, op=ALU.mult)
                TTp(out=t2, in0=gi, in1=ui, op=ALU.mult)
                TTv(out=hid_r[j], in0=t1, in1=t2, op=ALU.subtract)
                TTv(out=t1, in0=gr, in1=ui, op=ALU.mult)
                TTp(out=t2, in0=gi, in1=ur, op=ALU.mult)
                TTv(out=hid_i[j], in0=t1, in1=t2, op=ALU.add)
        if not NOFFN:
            # sentinel [P,1] zero tiles (bias operands) force each half-batch
            # of Sqrt ops to become ready only after the half's last qq is
            # written (qq adds are in-order on Pool, so qq[hi] done => all
            # done), and each Tanh half only after the half's last Sqrt. The
            # scheduler then runs each batch back-to-back on ACT: ~5 table
            # loads per iteration instead of one sqrt/exp pair per j tile.
            for half in range(2):
                lo, hi = half * 8, half * 8 + 7
                sent1 = p_sm.tile([P, 1], f32, name="sent1_%d" % half)
                TSv(out=sent1, in0=qs_[hi][:, 0:1], scalar1=0.0, scalar2=None,
                    op0=ALU.mult)
                for j in range(lo, hi + 1):   # batched Sqrt (one table window)
                    nc.scalar.activation(out=qs_[j], in_=qs_[j], func=AF.Sqrt,
                                         bias=sent1)
                sent2 = p_sm.tile([P, 1], f32, name="sent2_%d" % half)
                TSv(out=sent2, in0=qs_[hi][:, 0:1], scalar1=0.0, scalar2=None,
                    op0=ALU.mult)
                for j in range(lo, hi + 1):   # batched Tanh (exp/tanh set)
                    s_ = p_fh.tile([P, TQ], bf16, tag="f_s", bufs=16)
                    nc.scalar.activation(out=s_, in_=qs_[j], func=AF.Tanh, scale=0.5,
                                         bias=sent2)
                    TSv(out=s_, in0=s_, scalar1=0.5, scalar2=0.5, op0=ALU.mult,
                        op1=ALU.add)
                    TTv(out=hid_r[j], in0=hid_r[j], in1=s_, op=ALU.mult)
                    TTv(out=hid_i[j], in0=hid_i[j], in1=s_, op=ALU.mult)

        # ================= down proj (4-mult, kt-outer) + out =================
        es_ps.close()   # free the 6-bank pool; down needs 8 banks
        es_dn = ExitStack()
        ps_dn = es_dn.enter_context(tc.tile_pool(name="ps_dn", bufs=1, space="PSUM"))
        pd_r = [ps_dn.tile([P, TQ], f32, name='pdr%d' % ot) for ot in range(FT)]
        pd_i = [ps_dn.tile([P, TQ], f32, name='pdi%d' % ot) for ot in range(FT)]
        for kt in ([] if NOFFN else range(16)):
            w = p_fh.tile([P, 3 * D], bf16, tag="wdn", bufs=4, name="wdn", uniquify=True)
            if 'nowdma' not in ABLATE:
                nc.sync.dma_start(out=w, in_=io['wdn'][kt * P:(kt + 1) * P, :])
            st, sp = kt == 0, kt == 15
            for ot in range(FT):
                msl = slice(ot * P, (ot + 1) * P)
                nc.tensor.matmul(pd_r[ot], w[:, 0:D][:, msl], hid_r[kt], start=st, stop=False)
                nc.tensor.matmul(pd_r[ot], w[:, 2 * D:3 * D][:, msl], hid_i[kt],
                                 start=False, stop=sp)
                nc.tensor.matmul(pd_i[ot], w[:, D:2 * D][:, msl], hid_r[kt], start=st, stop=False)
                nc.tensor.matmul(pd_i[ot], w[:, 0:D][:, msl], hid_i[kt], start=False, stop=sp)
        for ot in range(FT):
            outr = p_keep.tile([P, TQ], f32, tag="ores", bufs=2)
            outi = p_keep.tile([P, TQ], f32, tag="ores", bufs=2)
            if NOFFN:
                TSv(out=outr, in0=res_r[ot], scalar1=1.0, scalar2=None, op0=ALU.mult)
                TSv(out=outi, in0=res_i[ot], scalar1=1.0, scalar2=None, op0=ALU.mult)
            else:
                TTv(out=outr, in0=pd_r[ot], in1=res_r[ot], op=ALU.add)
                TTv(out=outi, in0=pd_i[ot], in1=res_i[ot], op=ALU.add)
            nc.sync.dma_start(out=io['out_r'][ot * P:(ot + 1) * P, :], in_=outr)
            nc.sync.dma_start(out=io['out_i'][ot * P:(ot + 1) * P, :], in_=outi)
        es_dn.close()
        es_fh.close()
        es_keep.close()
        es_f.close()


def _build_module(n_iters=1, trivial_ln=False, trivial_b=True):
    import concourse.tile as tile
    from concourse import bacc, mybir

    f32 = mybir.dt.float32
    bf16 = mybir.dt.bfloat16
    nc = bacc.Bacc(None, target_bir_lowering=False, debug=False)
    with tile.TileContext(nc) as tc:
        with tc.tile_pool(name="dram", bufs=1, space="DRAM") as dram:
            io = {}

            def din(name, shape, dtype=f32):
                io[name] = dram.tile(shape, dtype, kind='ExternalInput', name=name,
                                     uniquify=False)

            din('xf_r', [D, S], bf16); din('xf_i', [D, S], bf16)
            din('wqkvo', [D, 4 * 3 * D], bf16)
            din('wgu', [D, 4 * 2 * 3 * TQ], bf16)
            din('wdn', [HID, 3 * D], bf16)
            din('cosk', [P, S], bf16); din('sink', [P, S], bf16)
            din('maskD', [P, 8 * P], bf16)
            din('shmat', [P, P], bf16)
            din('lnst', [1, 2 * 3 * TQ], bf16)
            if not trivial_ln:
                for ln in ['ln1', 'ln2']:
                    for q in ['gr', 'gi', 'br', 'bi']:
                        din(ln + '_' + q, [D])
            if not trivial_b:
                for nm in ['q', 'k', 'v', 'o']:
                    din('b_' + nm + '_r', [D]); din('b_' + nm + '_i', [D])
            io['out_r'] = dram.tile([D, TQ], f32, kind='ExternalOutput', name='out_r',
                                    uniquify=False)
            io['out_i'] = dram.tile([D, TQ], f32, kind='ExternalOutput', name='out_i',
                                    uniquify=False)

            if n_iters == 1:
                _emit_body(nc, tc, io, TRIV_LN=trivial_ln, TRIV_B=trivial_b)
            else:
                with tc.For_i(0, n_iters, 1):
                    _emit_body(nc, tc, io, TRIV_LN=trivial_ln, TRIV_B=trivial_b)
    nc.compile()
    return nc


def _prep_weights(full):
    """Host-side weight preprocessing -> packed bf16 arrays (shared by cores)."""
    import ml_dtypes
    bf = ml_dtypes.bfloat16

    def comps(lm, ph, karatsuba):
        mag = np.exp(lm.astype(np.float64))
        wr = (mag * np.cos(ph.astype(np.float64)))
        wi = (mag * np.sin(ph.astype(np.float64)))
        if karatsuba:
            return wr, wr + wi, wi - wr
        return wr, wi, -wi

    # qkvo: [D, 4, 3, D] (rows = in features, w.T layout)
    wqkvo = np.empty((D, 4, 3, D), dtype=bf)
    for m, nm in enumerate(['q', 'k', 'v', 'o']):
        kara = nm != 'v'
        c0, c1, c2 = comps(full[nm + '_lm'], full[nm + '_ph'], kara)
        wqkvo[:, m, 0, :] = c0.T.astype(bf)
        wqkvo[:, m, 1, :] = c1.T.astype(bf)
        wqkvo[:, m, 2, :] = c2.T.astype(bf)
    wqkvo = wqkvo.reshape(D, 4 * 3 * D)

    # gate/up: [D, G=4, 2, 3, 512]
    wgu = np.empty((D, 4, 2, 3, TQ), dtype=bf)
    for x, nm in enumerate(['gate', 'up']):
        c0, c1, c2 = comps(full[nm + '_lm'], full[nm + '_ph'], True)
        for g in range(4):
            osl = slice(g * TQ, (g + 1) * TQ)
            wgu[:, g, x, 0, :] = c0.T[:, osl].astype(bf)
            wgu[:, g, x, 1, :] = c1.T[:, osl].astype(bf)
            wgu[:, g, x, 2, :] = c2.T[:, osl].astype(bf)
    wgu = wgu.reshape(D, 4 * 2 * 3 * TQ)

    # down: [HID, 3, D]
    c0, c1, c2 = comps(full['down_lm'], full['down_ph'], False)
    wdn = np.empty((HID, 3, D), dtype=bf)
    wdn[:, 0, :] = c0.T.astype(bf)
    wdn[:, 1, :] = c1.T.astype(bf)
    wdn[:, 2, :] = c2.T.astype(bf)
    wdn = wdn.reshape(HID, 3 * D)

    # signed rotate-half permutation: out[m] = -pre[m+32] (m%64<32),
    #                                 out[m] = +pre[m-32] (m%64>=32)
    shmat = np.zeros((P, P), dtype=bf)
    for m in range(P):
        if m % HD < 32:
            shmat[m + 32, m] = -1.0
        else:
            shmat[m - 32, m] = 1.0
    return {'wqkvo': wqkvo, 'wgu': wgu, 'wdn': wdn, 'shmat': shmat}


def _host_inputs(x_real, x_imag, full, core, shared=None):
    import ml_dtypes
    bf = ml_dtypes.bfloat16
    if shared is None:
        shared = _prep_weights(full)
    b, hh = core // 2, core % 2
    own = np.arange(hh, S, 2)
    other = np.arange(1 - hh, S, 2)
    perm = np.concatenate([own, other])

    m = dict(shared)
    m['xf_r'] = np.ascontiguousarray(np.asarray(x_real)[b][perm].T.astype(bf))
    m['xf_i'] = np.ascontiguousarray(np.asarray(x_imag)[b][perm].T.astype(bf))

    xr64 = m['xf_r'].astype(np.float64)
    xi64 = m['xf_i'].astype(np.float64)
    lnst = np.empty((2, 3, TQ), dtype=np.float64)
    for ch in range(2):
        sl = slice(ch * TQ, (ch + 1) * TQ)
        mr = xr64[:, sl].mean(axis=0)
        mi = xi64[:, sl].mean(axis=0)
        var = (xr64[:, sl] ** 2 + xi64[:, sl] ** 2).mean(axis=0) - mr * mr - mi * mi
        iv = 1.0 / np.sqrt(var + EPS)
        lnst[ch, 0] = iv
        lnst[ch, 1] = mr * iv
        lnst[ch, 2] = mi * iv
    m['lnst'] = np.ascontiguousarray(lnst.reshape(1, 2 * 3 * TQ).astype(bf))

    invf = 1.0 / (10000.0 ** (np.arange(0, HD, 2, dtype=np.float64) / HD))
    pidx = (np.arange(P) % HD) % 32
    ang = perm[None, :].astype(np.float64) * invf[pidx][:, None]
    m['cosk'] = np.cos(ang).astype(bf)
    m['sink'] = np.sin(ang).astype(bf)

    # diagonal-block masks [P, 8*128]
    maskD = np.zeros((P, 8 * P), dtype=bf)
    for t in range(8):
        t4 = t % 4
        kg = 2 * (t4 * P + np.arange(P)) + (hh if t < 4 else 1 - hh)
        qg = 2 * (t4 * P + np.arange(P)) + hh
        maskD[:, t * P:(t + 1) * P] = (kg[:, None] <= qg[None, :]).astype(bf)
    m['maskD'] = maskD

    trivial_ln = _is_trivial_ln(full)
    if not trivial_ln:
        for ln in ['ln1', 'ln2']:
            for q in ['gr', 'gi', 'br', 'bi']:
                m[ln + '_' + q] = np.asarray(full[ln + '_' + q], dtype=np.float32)
    trivial_b = _is_trivial_b(full)
    if not trivial_b:
        for nm in ['q', 'k', 'v', 'o']:
            bm = np.asarray(full[nm + '_bm'], dtype=np.float64)
            bp = np.asarray(full[nm + '_bp'], dtype=np.float64)
            m['b_' + nm + '_r'] = (bm * np.cos(bp)).astype(np.float32)
            m['b_' + nm + '_i'] = (bm * np.sin(bp)).astype(np.float32)
    return m


def _is_trivial_ln(full):
    return (np.all(np.asarray(full['ln1_gr']) == 1) and np.all(np.asarray(full['ln2_gr']) == 1)
            and all(np.all(np.asarray(full[k]) == 0)
                    for k in ['ln1_gi', 'ln1_br', 'ln1_bi', 'ln2_gi', 'ln2_br', 'ln2_bi']))


def _is_trivial_b(full):
    return all(np.all(np.asarray(full[nm + '_bm']) == 0) for nm in ['q', 'k', 'v', 'o'])


def kernel(**inputs):
    from concourse.bass_utils import run_bass_kernel_spmd

    full = {k: np.asarray(v) for k, v in inputs.items()}
    x_real, x_imag = full['x_real'], full['x_imag']

    trivial_ln = _is_trivial_ln(full)
    trivial_b = _is_trivial_b(full)
    key = ('mod', trivial_ln, trivial_b)
    if key not in _CACHE:
        _CACHE[key] = _build_module(1, trivial_ln=trivial_ln, trivial_b=trivial_b)
    nc = _CACHE[key]

    shared = _prep_weights(full)
    in_maps = [_host_inputs(x_real, x_imag, full, c, shared) for c in range(NCORES)]
    res = run_bass_kernel_spmd(nc, in_maps, core_ids=list(range(NCORES)), trace=False)

    out = np.empty((2, B, S, D), dtype=np.float32)
    for c in range(NCORES):
        b, hh = c // 2, c % 2
        out[0, b, hh::2, :] = res.results[c]['out_r'].T
        out[1, b, hh::2, :] = res.results[c]['out_i'].T
    return out



# revision 17
# speedup vs baseline: 1.1253x; 1.1253x over previous
"""Trainium2 Bass kernel: complex-valued transformer block (nn_EqModelComplex).

v2 design:
- 8 cores = (batch b in 0..3) x (query-parity hh in 0..1). Each core owns the
  512 queries with global index == hh (mod 2) of one batch element. Host
  permutes the token axis per core to [own queries | other parity], so the
  causal structure of every attention tile is IDENTICAL on all cores
  (triangular diagonal blocks), and the LN1 output for chunk 0 doubles as the
  Q-path input. No collectives.
- All weight preprocessing (exp/cos/sin of the magnitude-phase params) is done
  on the host in numpy; the device receives packed bf16 weight components.
  Q/K/O/gate/up use the 3-multiply Karatsuba complex form
  (m1=wr@(xr+xi), m2=(wr+wi)@xi, m3=(wi-wr)@xr; yr=m1-m2, yi=m1+m3);
  V (tokens-stationary) and down use the plain 4-multiply form with a host
  negated component.
- Feature-major layout: SBUF tiles are (features on partitions, tokens free).
  All matmuls in bf16 (x arrives bf16 from the host). LN stats via bf16
  ones-matmuls; the per-token stat / softmax-denominator broadcasts use gpsimd
  partition_broadcast (Pool engine). LN stats chains are split so a chunk's
  reduction hides under the previous chunk's projections.
- RoPE: rotate-half done on the PE as a constant signed-permutation matmul
  (shmat), leaving 3 bf16 DVE ops per rotation.
- Attention: transposed scores (tk on partitions, tq free), score/exp/AV all
  column-trimmed to the causal region, exp (no max subtraction) straight to
  bf16, multiplicative diagonal-block mask, denominators via a ones-column in
  V; score/exp/mask pipelined LAG tiles ahead of the AV accumulation.
- Activation-table discipline: sqrt is the only function outside the
  exp/tanh/square/copy table set, so the FFN nonlinearity is computed in two
  stages with sentinel-gated half-batches of 8 Sqrt and 8 Tanh ops, keeping
  the steady-state table-load count at ~5/iteration instead of ~27.
- Attention postamble: the single-buffered po psum banks are drained to SBUF
  by two immediate DVE copies so the next head's AV accumulation (strict FIFO
  on PE) is unblocked; recip/broadcast/scale then run off the critical path.
"""
import sys, os
sys.path.insert(0, '/opt/trn_rl_repo')
import math
import numpy as np
from contextlib import ExitStack

P = 128
D = 512
S = 1024
B = 4
H = 8
HD = 64
HID = 2048
TQ = 512
FT = D // P          # 4
NCORES = 8
EPS = 1e-6
SCALE = 1.0 / math.sqrt(HD)

_CACHE = {}
ABLATE = set()   # timing-ablation flags: 'noatt', 'noffn', 'nowdma'


def _emit_body(nc, tc, io, TRIV_LN=False, TRIV_B=False):
    from concourse import mybir

    dt = mybir.dt
    AF = mybir.ActivationFunctionType
    ALU = mybir.AluOpType
    f32 = dt.float32
    f32r = dt.float32r
    bf16 = dt.bfloat16
    TTv = nc.vector.tensor_tensor      # DVE
    TTp = nc.gpsimd.tensor_tensor      # Pool
    TSv = nc.vector.tensor_scalar
    STTv = nc.vector.scalar_tensor_tensor

    ctx = ExitStack()
    with ctx:
        # ---------------- long-lived pools ----------------
        const = ctx.enter_context(tc.tile_pool(name="const", bufs=1))
        p_tmp = ctx.enter_context(tc.tile_pool(name="p_tmp", bufs=1))
        p_sm = ctx.enter_context(tc.tile_pool(name="p_sm", bufs=1))
        p_bc = ctx.enter_context(tc.tile_pool(name="p_bc", bufs=1))

        es_ps = ExitStack()
        ps = es_ps.enter_context(tc.tile_pool(name="ps", bufs=6, space="PSUM"))

        ones_b = const.tile([P, 1], bf16)
        nc.vector.memset(ones_b, 1.0)
        shmat = const.tile([P, P], bf16, name='shmat')
        nc.sync.dma_start(out=shmat, in_=io['shmat'][:])
        ceps = const.tile([1, 1], f32)
        nc.vector.memset(ceps, EPS)

        lncols = {}
        if not TRIV_LN:
            for key in ['ln1_gr', 'ln1_gi', 'ln1_br', 'ln1_bi',
                        'ln2_gr', 'ln2_gi', 'ln2_br', 'ln2_bi']:
                c = const.tile([P, FT], f32, name='c_' + key)
                nc.sync.dma_start(out=c, in_=io[key].rearrange("(t p) -> p t", p=P))
                lncols[key] = c
        bcols = {}
        if not TRIV_B:
            for nm in ['q', 'k', 'v', 'o']:
                br = const.tile([P, FT], f32, name='cb_r_' + nm)
                bi = const.tile([P, FT], f32, name='cb_i_' + nm)
                nc.sync.dma_start(out=br, in_=io['b_' + nm + '_r'].rearrange("(t p) -> p t", p=P))
                nc.sync.dma_start(out=bi, in_=io['b_' + nm + '_i'].rearrange("(t p) -> p t", p=P))
                bcols[nm] = (br, bi)

        # x load (full permuted sequence, feature-major)
        es_x = ExitStack()
        p_x = es_x.enter_context(tc.tile_pool(name="p_x", bufs=1, side='right'))
        xf_r = [p_x.tile([P, S], bf16, name='xfr%d' % kt) for kt in range(FT)]
        xf_i = [p_x.tile([P, S], bf16, name='xfi%d' % kt) for kt in range(FT)]
        es_wq = ExitStack()
        p_wq = es_wq.enter_context(tc.tile_pool(name="p_wq", bufs=1, side='right'))
        wqkvo = [p_wq.tile([P, 4 * 3 * D], bf16, name='wqkvo%d' % kt) for kt in range(FT)]
        ck = const.tile([P, S], bf16, name='ck')
        sk = const.tile([P, S], bf16, name='sk')
        maskD = const.tile([P, 8 * P], bf16, name='maskD')

        def dma_x(ch):
            csl = slice(ch * TQ, (ch + 1) * TQ)
            for kt in range(FT):
                nc.sync.dma_start(out=xf_r[kt][:, csl], in_=io['xf_r'][kt * P:(kt + 1) * P, csl])
                nc.sync.dma_start(out=xf_i[kt][:, csl], in_=io['xf_i'][kt * P:(kt + 1) * P, csl])

        def dma_w(m):
            if 'nowdma' in ABLATE:
                return
            ms = slice(m * 3 * D, (m + 1) * 3 * D)
            for kt in range(FT):
                nc.sync.dma_start(out=wqkvo[kt][:, ms],
                                  in_=io['wqkvo'][kt * P:(kt + 1) * P, ms])

        # host-computed LN1 stats: per chunk [iv | mr*iv | mi*iv]
        lnst_t = const.tile([1, 2 * 3 * TQ], bf16, name='lnst_t')
        nc.sync.dma_start(out=lnst_t, in_=io['lnst'][:])

        # DMA order = first-consumer order: x ch0 (LN1), k weights, x ch1,
        # rope tables, then q/v/o weights
        dma_x(0)
        dma_w(1)
        dma_x(1)
        nc.sync.dma_start(out=ck, in_=io['cosk'][:])
        nc.sync.dma_start(out=sk, in_=io['sink'][:])
        dma_w(0)
        dma_w(2)
        dma_w(3)
        nc.sync.dma_start(out=maskD, in_=io['maskD'][:])

        def wsl(kt, m, c):
            # slice for matrix m (0=q,1=k,2=v,3=o) component c
            base = (m * 3 + c) * D
            return wqkvo[kt][:, base:base + D]

        # ---------- layernorm (NT tokens, feature-major), split in two ----------
        # ln_stats emits the reduction + broadcast chain; ln_normalize consumes
        # the broadcast tiles. Splitting lets a later chunk's stats chain hide
        # under the previous chunk's projections.
        # xbf=True means the x_r/x_i APs are already bf16 (host-quantized x);
        # otherwise (f32 residuals) bf16 staging copies feed the stat matmuls.
        def ln_stats(x_r, x_i, xbf=False):
            NT = x_r[0].shape[-1]
            s_ps = [ps.tile([1, NT], f32, tag="ps", name="lnps%d" % q) for q in range(3)]
            for kt in range(FT):
                if xbf:
                    xrb, xib = x_r[kt], x_i[kt]
                else:
                    xrb = p_tmp.tile([P, NT], bf16, tag="ln_xrb", bufs=2)
                    xib = p_tmp.tile([P, NT], bf16, tag="ln_xib", bufs=2)
                    nc.gpsimd.tensor_copy(out=xrb, in_=x_r[kt])
                    nc.scalar.copy(out=xib, in_=x_i[kt])
                ta = p_tmp.tile([P, NT], bf16, tag="ln_tab", bufs=2)
                tb = p_tmp.tile([P, NT], bf16, tag="ln_tbb", bufs=2)
                STTv(out=ta, in0=x_r[kt], scalar=1.0, in1=x_r[kt], op0=ALU.mult,
                     op1=ALU.mult)
                STTv(out=tb, in0=x_i[kt], scalar=1.0, in1=x_i[kt], op0=ALU.mult,
                     op1=ALU.mult)
                st, sp = kt == 0, kt == FT - 1
                nc.tensor.matmul(s_ps[0], ones_b, xrb, start=st, stop=sp)
                nc.tensor.matmul(s_ps[1], ones_b, xib, start=st, stop=sp)
                nc.tensor.matmul(s_ps[2], ones_b, ta, start=st, stop=False)
                nc.tensor.matmul(s_ps[2], ones_b, tb, start=False, stop=sp)
            # small [1,NT] ops — all on DVE (except the Sqrt, which only ACT
            # has) so the chain doesn't pay a cross-engine semaphore hop per op
            mr = p_sm.tile([1, NT], f32, tag="sm_mr", bufs=1)
            mi = p_sm.tile([1, NT], f32, tag="sm_mi", bufs=1)
            vv = p_sm.tile([1, NT], f32, tag="sm_vv", bufs=1)
            t2 = p_sm.tile([1, NT], f32, tag="sm_t2", bufs=1)
            TSv(out=mr, in0=s_ps[0], scalar1=1.0 / D, scalar2=None, op0=ALU.mult)
            TSv(out=mi, in0=s_ps[1], scalar1=1.0 / D, scalar2=None, op0=ALU.mult)
            TTv(out=t2, in0=mr, in1=mr, op=ALU.mult)
            STTv(out=vv, in0=s_ps[2], scalar=1.0 / D, in1=t2, op0=ALU.mult, op1=ALU.subtract)
            TTv(out=t2, in0=mi, in1=mi, op=ALU.mult)
            TTv(out=vv, in0=vv, in1=t2, op=ALU.subtract)
            sm3 = p_sm.tile([1, 3 * NT], bf16, tag="sm_sm3", bufs=2)
            nc.scalar.activation(out=vv, in_=vv, func=AF.Sqrt, bias=ceps)
            nc.vector.reciprocal(out=vv, in_=vv)                      # iv
            bc3 = p_bc.tile([P, 3 * NT], bf16, tag="bc_3", bufs=2)
            # iv broadcasts first: the normalize's leading multiply only needs
            # iv, so the mean broadcasts come off the critical path
            nc.vector.tensor_copy(out=sm3[:, 0:NT], in_=vv)
            nc.gpsimd.partition_broadcast(bc3[:, 0:NT], sm3[:, 0:NT])
            TTv(out=sm3[:, NT:2 * NT], in0=mr, in1=vv, op=ALU.mult)   # mr*iv
            TTv(out=sm3[:, 2 * NT:3 * NT], in0=mi, in1=vv, op=ALU.mult)
            nc.gpsimd.partition_broadcast(bc3[:, NT:3 * NT], sm3[:, NT:3 * NT])
            return bc3[:, 0:NT], bc3[:, NT:2 * NT], bc3[:, 2 * NT:3 * NT]

        def ln_normalize(bc3, x_r, x_i, ln, dst_r, dst_i, dst_s, hpool, htag, hbufs):
            NT = x_r[0].shape[-1]
            iv_bc, mr_bc, mi_bc = bc3
            gcols = None if TRIV_LN else (lncols[ln + '_gr'], lncols[ln + '_gi'],
                                          lncols[ln + '_br'], lncols[ln + '_bi'])
            for kt in range(FT):
                tr = p_tmp.tile([P, NT], bf16, tag="ln_tr", bufs=2)
                ti = p_tmp.tile([P, NT], bf16, tag="ln_ti", bufs=2)
                idx = len(dst_r)
                if hbufs == 0:
                    hr = hpool.tile([P, NT], bf16, name=htag + "hr%d" % idx, uniquify=True)
                    hi = hpool.tile([P, NT], bf16, name=htag + "hi%d" % idx, uniquify=True)
                    hs = hpool.tile([P, NT], bf16, name=htag + "hs%d" % idx,
                                    uniquify=True)
                else:
                    hr = hpool.tile([P, NT], bf16, tag=htag + "r", bufs=hbufs,
                                    name=htag + "hr", uniquify=True)
                    hi = hpool.tile([P, NT], bf16, tag=htag + "i", bufs=hbufs,
                                    name=htag + "hi", uniquify=True)
                    hs = hpool.tile([P, NT], bf16, tag=htag + "s", bufs=hbufs,
                                    name=htag + "hs", uniquify=True)
                TTv(out=tr, in0=x_r[kt], in1=iv_bc, op=ALU.mult)
                TTv(out=ti, in0=x_i[kt], in1=iv_bc, op=ALU.mult)
                if TRIV_LN:
                    TTv(out=hr, in0=tr, in1=mr_bc, op=ALU.subtract)
                    TTp(out=hi, in0=ti, in1=mi_bc, op=ALU.subtract)
                else:
                    nr = p_tmp.tile([P, NT], f32, tag="ln_nr", bufs=2)
                    ni = p_tmp.tile([P, NT], f32, tag="ln_ni", bufs=2)
                    TTp(out=nr, in0=tr, in1=mr_bc, op=ALU.subtract)
                    TTp(out=ni, in0=ti, in1=mi_bc, op=ALU.subtract)
                    grc, gic, brc, bic = gcols
                    ta = p_tmp.tile([P, NT], f32, tag="ln_ta", bufs=2)
                    tb = p_tmp.tile([P, NT], f32, tag="ln_tb", bufs=2)
                    TSv(out=ta, in0=nr, scalar1=grc[:, kt:kt + 1], scalar2=None, op0=ALU.mult)
                    TSv(out=tb, in0=ni, scalar1=gic[:, kt:kt + 1], scalar2=None, op0=ALU.mult)
                    TTv(out=ta, in0=ta, in1=tb, op=ALU.subtract)
                    TSv(out=hr, in0=ta, scalar1=brc[:, kt:kt + 1], scalar2=None, op0=ALU.add)
                    TSv(out=ta, in0=nr, scalar1=gic[:, kt:kt + 1], scalar2=None, op0=ALU.mult)
                    TSv(out=tb, in0=ni, scalar1=grc[:, kt:kt + 1], scalar2=None, op0=ALU.mult)
                    TTv(out=ta, in0=ta, in1=tb, op=ALU.add)
                    TSv(out=hi, in0=ta, scalar1=bic[:, kt:kt + 1], scalar2=None, op0=ALU.add)
                TTv(out=hs, in0=hr, in1=hi, op=ALU.add)
                dst_r.append(hr)
                dst_i.append(hi)
                dst_s.append(hs)

        # Karatsuba complex matmul accumulation (weights stationary)
        def kmm(ps3, w3, kt, nkt, rhs_r, rhs_i, rhs_s, msl):
            m1, m2, m3 = ps3
            wr, wpw, wmw = w3
            st, sp = kt == 0, kt == nkt - 1
            nc.tensor.matmul(m1, wr[:, msl], rhs_s, start=st, stop=sp)
            nc.tensor.matmul(m2, wpw[:, msl], rhs_i, start=st, stop=sp)
            nc.tensor.matmul(m3, wmw[:, msl], rhs_r, start=st, stop=sp)

        # rope: dst (bf16) <- pre*cos + rothalf(pre)*sin. rothalf is a signed
        # partition permutation, done as a PE matmul with the constant shmat;
        # the PSUM result goes through an ACT copy, leaving 3 bf16 DVE ops.
        def rope(dst, pre, cosT, sinT):
            NT = pre.shape[-1]
            shps = ps.tile([P, NT], f32, tag="ps", name="shps")
            nc.tensor.matmul(shps, shmat, pre, start=True, stop=True)
            shb = p_kq.tile([P, NT], bf16, tag="rope_shb", bufs=2)
            nc.scalar.copy(out=shb, in_=shps)
            tmp = p_kq.tile([P, NT], bf16, tag="rope_tmp", bufs=2)
            TTv(out=dst, in0=pre, in1=cosT, op=ALU.mult)
            TTv(out=tmp, in0=shb, in1=sinT, op=ALU.mult)
            TTv(out=dst, in0=dst, in1=tmp, op=ALU.add)

        # Karatsuba combine: yr = m1-m2, yi = m1+m3. TensorTensor cannot read
        # two PSUM banks, so m1 goes through SBUF via one ACT copy first.
        def kcombine(dst_r, dst_i, m3_, bias_r=None, bias_i=None):
            m1sb = p_tmp.tile([P, dst_r.shape[-1]], f32, tag="m1sb", bufs=2)
            nc.scalar.copy(out=m1sb, in_=m3_[0])
            if bias_r is None:
                TTv(out=dst_r, in0=m1sb, in1=m3_[1], op=ALU.subtract)
                TTv(out=dst_i, in0=m1sb, in1=m3_[2], op=ALU.add)
            else:
                t = p_tmp.tile([P, dst_r.shape[-1]], f32, tag="cmb_t", bufs=2)
                TTv(out=t, in0=m1sb, in1=m3_[1], op=ALU.subtract)
                TSv(out=dst_r, in0=t, scalar1=bias_r, scalar2=None, op0=ALU.add)
                TTv(out=t, in0=m1sb, in1=m3_[2], op=ALU.add)
                TSv(out=dst_i, in0=t, scalar1=bias_i, scalar2=None, op0=ALU.add)

        # ===== pools for attention-era tiles (right stack: p_am under p_kq
        # so the bulky k/q/v/h1 tiles free right after the attention loop
        # while attn tiles survive into the O projection) =====
        es_am = ExitStack()
        p_am = es_am.enter_context(tc.tile_pool(name="p_am", bufs=1, side='right'))
        attn_r = [p_am.tile([P, TQ], bf16, name='attnr%d' % ot) for ot in range(FT)]
        attn_i = [p_am.tile([P, TQ], bf16, name='attni%d' % ot) for ot in range(FT)]
        attn_s = [p_am.tile([P, TQ], bf16, name='attns%d' % ot) for ot in range(FT)]
        es_att = ExitStack()
        p_kq = es_att.enter_context(tc.tile_pool(name="p_kq", bufs=1, side='right'))
        k_rot_r = [p_kq.tile([P, S], bf16, name='krr%d' % ot) for ot in range(FT)]
        k_rot_i = [p_kq.tile([P, S], bf16, name='kri%d' % ot) for ot in range(FT)]
        q_rot_r = [p_kq.tile([P, TQ], bf16, name='qrr%d' % ot) for ot in range(FT)]
        q_rot_i = [p_kq.tile([P, TQ], bf16, name='qri%d' % ot) for ot in range(FT)]
        vaug = [p_kq.tile([P, H, 129], bf16, name='vaug%d' % t) for t in range(8)]
        for t in range(8):
            nc.gpsimd.memset(vaug[t][:, :, 64:65], 1.0)

        bk = bcols.get('k', (None, None)) if not TRIV_B else (None, None)
        bq = bcols.get('q', (None, None)) if not TRIV_B else (None, None)

        def kq_proj(m, ot, hoff, msl):
            m3_ = [ps.tile([P, TQ], f32, tag="ps", name="kqps%d" % q) for q in range(3)]
            for kt in range(FT):
                kmm(m3_, (wsl(kt, m, 0), wsl(kt, m, 1), wsl(kt, m, 2)), kt, FT,
                    h1_r[hoff + kt], h1_i[hoff + kt], h1_s[hoff + kt], msl)
            pre_r = p_kq.tile([P, TQ], bf16, tag="pre_r", bufs=2)
            pre_i = p_kq.tile([P, TQ], bf16, tag="pre_i", bufs=2)
            bb = bq if m == 0 else bk
            bcr = None if TRIV_B else bb[0][:, ot:ot + 1]
            bci = None if TRIV_B else bb[1][:, ot:ot + 1]
            kcombine(pre_r, pre_i, m3_, bcr, bci)
            return pre_r, pre_i

        # ===== LN1 ch0 -> [ch1 stats] -> K-ch0 + Q -> ch1 normalize -> K-ch1 =====
        # (chunk 1's stats/broadcast chain hides under the chunk-0 projections)
        h1_r, h1_i, h1_s = [], [], []
        xs = []
        for ch in range(2):
            csl = slice(ch * TQ, (ch + 1) * TQ)
            xs.append(([xf_r[kt][:, csl] for kt in range(FT)],
                       [xf_i[kt][:, csl] for kt in range(FT)]))
        def ln1_bc(ch):
            bc3 = p_bc.tile([P, 3 * TQ], bf16, tag="bc_3", bufs=2)
            nc.gpsimd.partition_broadcast(bc3, lnst_t[:, ch * 1536:(ch + 1) * 1536])
            return bc3[:, 0:TQ], bc3[:, TQ:2 * TQ], bc3[:, 2 * TQ:3 * TQ]

        NOATT = 'noatt' in ABLATE
        bc0 = ln1_bc(0)
        if not NOATT:
            ln_normalize(bc0, xs[0][0], xs[0][1], 'ln1', h1_r, h1_i, h1_s, p_kq, "h1", 0)
        bc1 = ln1_bc(1)
        for ch in ([] if NOATT else range(2)):
            csl = slice(ch * TQ, (ch + 1) * TQ)
            if ch == 1:
                ln_normalize(bc1, xs[1][0], xs[1][1], 'ln1', h1_r, h1_i, h1_s,
                             p_kq, "h1", 0)
            for ot in range(FT):
                msl = slice(ot * P, (ot + 1) * P)
                pre_r, pre_i = kq_proj(1, ot, ch * FT, msl)
                rope(k_rot_r[ot][:, csl], pre_r, ck[:, csl], sk[:, csl])
                rope(k_rot_i[ot][:, csl], pre_i, ck[:, csl], sk[:, csl])
                if ch == 0:
                    pre_r, pre_i = kq_proj(0, ot, 0, msl)
                    rope(q_rot_r[ot], pre_r, ck[:, 0:TQ], sk[:, 0:TQ])
                    rope(q_rot_i[ot], pre_i, ck[:, 0:TQ], sk[:, 0:TQ])

        # ================= V proj (tokens stationary, 4-mult) =================
        for t in ([] if NOATT else range(8)):
            ch, t4 = t // 4, t % 4
            tsl = slice(t4 * P, (t4 + 1) * P)
            pvr = ps.tile([P, D], f32, tag="ps", name="pvr")
            pvi = ps.tile([P, D], f32, tag="ps", name="pvi")
            for kt in range(FT):
                st, sp = kt == 0, kt == FT - 1
                hr_t, hi_t = h1_r[ch * FT + kt], h1_i[ch * FT + kt]
                nc.tensor.matmul(pvr, hr_t[:, tsl], wsl(kt, 2, 0), start=st, stop=False)
                nc.tensor.matmul(pvr, hi_t[:, tsl], wsl(kt, 2, 2), start=False, stop=sp)
                nc.tensor.matmul(pvi, hr_t[:, tsl], wsl(kt, 2, 1), start=st, stop=False)
                nc.tensor.matmul(pvi, hi_t[:, tsl], wsl(kt, 2, 0), start=False, stop=sp)
            nc.scalar.copy(out=vaug[t][:, :, 0:64], in_=pvr.rearrange("p (h d) -> p h d", h=H))
            nc.vector.tensor_copy(out=vaug[t][:, :, 65:129],
                                  in_=pvi.rearrange("p (h d) -> p h d", h=H))

        # ================= attention =================
        es_at = ExitStack()
        ps_ar = es_at.enter_context(tc.tile_pool(name="ps_ar", bufs=1, space="PSUM"))
        ps_ai = es_at.enter_context(tc.tile_pool(name="ps_ai", bufs=1, space="PSUM"))
        bv = bcols.get('v', (None, None)) if not TRIV_B else (None, None)
        LAG = 5
        if NOATT:
            for ot in range(FT):
                nc.vector.memset(attn_r[ot], 0.01)
                nc.vector.memset(attn_i[ot], 0.01)
                nc.vector.memset(attn_s[ot], 0.02)
        for h in ([] if NOATT else range(H)):
            ot, prow = h // 2, 64 * (h % 2)
            po_r = ps_ar.tile([65, TQ], f32, tag="at_r")
            po_i = ps_ai.tile([64, TQ], f32, tag="at_i")
            p32s = {}
            # software pipeline: emit score/exp/mask LAG tiles ahead of the AV
            # accumulation so the PE never stalls on the exp/mask latency
            for step in range(8 + LAG):
                if step < 8:
                    t = step
                    c0 = 128 * (t % 4)
                    tsl = slice(t * P, (t + 1) * P)
                    csl = slice(c0, TQ)
                    pst = ps.tile([P, TQ], f32, tag="ps", name="pst")
                    nc.tensor.matmul(pst[:, csl], k_rot_r[ot][prow:prow + 64, tsl],
                                     q_rot_r[ot][prow:prow + 64, csl], start=True, stop=False)
                    nc.tensor.matmul(pst[:, csl], k_rot_i[ot][prow:prow + 64, tsl],
                                     q_rot_i[ot][prow:prow + 64, csl], start=False, stop=True)
                    p32 = p_am.tile([P, TQ], bf16, tag="p32", bufs=LAG + 1)
                    nc.scalar.activation(out=p32[:, csl], in_=pst[:, csl], func=AF.Exp,
                                         scale=SCALE)
                    TTv(out=p32[:, c0:c0 + 128], in0=p32[:, c0:c0 + 128],
                        in1=maskD[:, t * P:(t + 1) * P], op=ALU.mult)
                    p32s[t] = p32
                if step >= LAG:
                    t = step - LAG
                    c0 = 128 * (t % 4)
                    csl = slice(c0, TQ)
                    p32 = p32s.pop(t)
                    # accumulate only the causal column range; columns < c0 get
                    # no contribution from this tile (masked to zero anyway)
                    nc.tensor.matmul(po_r[:, csl], vaug[t][:, h, 0:65], p32[:, csl],
                                     start=(t == 0), stop=(t == 7), skip_group_check=True)
                    nc.tensor.matmul(po_i[:, csl], vaug[t][:, h, 65:129], p32[:, csl],
                                     start=(t == 0), stop=(t == 7), skip_group_check=True)
            # Drain the single-buffered po psum banks IMMEDIATELY with two DVE
            # copies so the next head's AV accumulation (same banks, strict
            # FIFO on PE) is unblocked; the recip/broadcast/scale chain then
            # runs on the SBUF copies off the PE-critical path.
            sb_r = p_am.tile([65, TQ], bf16, tag="po_sbr", bufs=1)
            sb_i = p_am.tile([64, TQ], bf16, tag="po_sbi", bufs=1)
            nc.vector.tensor_copy(out=sb_r, in_=po_r)
            nc.vector.tensor_copy(out=sb_i, in_=po_i)
            rec = p_sm.tile([1, TQ], f32, tag="sm_rec", bufs=2)
            nc.vector.reciprocal(out=rec, in_=sb_r[64:65, :])
            rec_bc = p_bc.tile([64, TQ], f32, tag="bc_rec", bufs=1)
            nc.gpsimd.partition_broadcast(rec_bc, rec)
            ar_sl = attn_r[ot][prow:prow + 64, :]
            ai_sl = attn_i[ot][prow:prow + 64, :]
            TTv(out=ar_sl, in0=sb_r[0:64, :], in1=rec_bc, op=ALU.mult)
            TTv(out=ai_sl, in0=sb_i, in1=rec_bc, op=ALU.mult)
            if not TRIV_B:
                TSv(out=ar_sl, in0=ar_sl, scalar1=bv[0][prow:prow + 64, ot:ot + 1],
                    scalar2=None, op0=ALU.add)
                TSv(out=ai_sl, in0=ai_sl, scalar1=bv[1][prow:prow + 64, ot:ot + 1],
                    scalar2=None, op0=ALU.add)
            if h % 2 == 1:   # both heads of this ot tile done
                TTv(out=attn_s[ot], in0=attn_r[ot], in1=attn_i[ot], op=ALU.add)
        es_at.close()
        es_att.close()   # k_rot/q_rot/vaug dead after the attention loop

        # FFN gate/up weight pool: group 0 streams in during late attention,
        # group 1 during the O projection + LN2 window.
        es_f = ExitStack()
        p_fw = es_f.enter_context(tc.tile_pool(name="p_fw", bufs=1))
        wgu_t = {}

        def load_wgu(g):
            tl = []
            for kt in range(FT):
                w = p_fw.tile([P, 2 * 3 * TQ], bf16, tag="wgu", bufs=4, name="wgu", uniquify=True)
                if 'nowdma' not in ABLATE:
                    nc.sync.dma_start(out=w, in_=io['wgu'][kt * P:(kt + 1) * P,
                                                           g * 3072:(g + 1) * 3072])
                tl.append(w)
            wgu_t[g] = tl

        NOFFN = 'noffn' in ABLATE
        if not NOFFN:
            load_wgu(0)

        # ================= O proj + residual =================
        es_keep = ExitStack()
        p_keep = es_keep.enter_context(tc.tile_pool(name="p_keep", bufs=1))
        res_r = [p_keep.tile([P, TQ], f32, name='resr%d' % ot) for ot in range(FT)]
        res_i = [p_keep.tile([P, TQ], f32, name='resi%d' % ot) for ot in range(FT)]
        resb_r = [p_keep.tile([P, TQ], bf16, name='resbr%d' % ot) for ot in range(FT)]
        resb_i = [p_keep.tile([P, TQ], bf16, name='resbi%d' % ot) for ot in range(FT)]

        if not NOFFN:
            load_wgu(1)
        xq_r = [xf_r[ot][:, 0:TQ] for ot in range(FT)]
        xq_i = [xf_i[ot][:, 0:TQ] for ot in range(FT)]
        bo = bcols.get('o', (None, None)) if not TRIV_B else (None, None)
        for ot in range(FT):
            msl = slice(ot * P, (ot + 1) * P)
            m3_ = [ps.tile([P, TQ], f32, tag="ps", name="ops%d" % q) for q in range(3)]
            for kt in range(FT):
                kmm(m3_, (wsl(kt, 3, 0), wsl(kt, 3, 1), wsl(kt, 3, 2)), kt, FT,
                    attn_r[kt], attn_i[kt], attn_s[kt], msl)
            tr = p_keep.tile([P, TQ], f32, tag="ores", bufs=2)
            ti = p_keep.tile([P, TQ], f32, tag="ores", bufs=2)
            bcr = None if TRIV_B else bo[0][:, ot:ot + 1]
            bci = None if TRIV_B else bo[1][:, ot:ot + 1]
            kcombine(tr, ti, m3_, bcr, bci)
            TTp(out=res_r[ot], in0=tr, in1=xq_r[ot], op=ALU.add)
            TTp(out=res_i[ot], in0=ti, in1=xq_i[ot], op=ALU.add)
            TTv(out=resb_r[ot], in0=tr, in1=xq_r[ot], op=ALU.add)
            TTv(out=resb_i[ot], in0=ti, in1=xq_i[ot], op=ALU.add)
        es_am.close()
        es_wq.close()
        es_x.close()   # xf no longer needed after the residual
        es_fh = ExitStack()
        p_fh = es_fh.enter_context(tc.tile_pool(name="p_fh", bufs=1))

        # ================= LN2 =================
        h2_r, h2_i, h2_s = [], [], []
        if not NOFFN:
            bc2 = ln_stats(resb_r, resb_i, xbf=True)
            ln_normalize(bc2, resb_r, resb_i, 'ln2', h2_r, h2_i, h2_s, p_keep, "h2", 0)


        # ================= FFN gate/up -> hid =================
        # hid is computed in two stages so that ALL 16 Sqrt ops and ALL 16
        # Tanh ops form contiguous ACT batches (sqrt is the only function
        # outside the exp/tanh/square/copy table set, so each sqrt<->tanh
        # alternation in the final ACT order costs a ~1.3-2.7us table load):
        #   stage 1 (per group): gate/up matmuls+combines, qq=|g|^2, and the
        #     UNGATED complex product hidp = g*u  (g,u short-lived)
        #   stage 2: batched sqrt(qq), batched tanh, then hid = s*hidp in place
        hid_r = [p_fh.tile([P, TQ], bf16, name='hidr%d' % j) for j in range(16)]
        hid_i = [p_fh.tile([P, TQ], bf16, name='hidi%d' % j) for j in range(16)]
        qs_ = [p_fh.tile([P, TQ], bf16, name='qq%d' % j) for j in range(16)]
        for g in ([] if NOFFN else range(4)):
            if g + 2 < 4:
                load_wgu(g + 2)
            gtiles = wgu_t.pop(g)
            for jj in range(4):
                j = g * 4 + jj
                jsl = slice(jj * P, (jj + 1) * P)
                mg = [ps.tile([P, TQ], f32, tag="ps", name="gps%d" % q) for q in range(3)]
                for kt in range(FT):
                    w3 = (gtiles[kt][:, 0:TQ], gtiles[kt][:, TQ:2 * TQ],
                          gtiles[kt][:, 2 * TQ:3 * TQ])
                    kmm(mg, w3, kt, FT, h2_r[kt], h2_i[kt], h2_s[kt], jsl)
                gr = p_fh.tile([P, TQ], bf16, tag="f_gr", bufs=2)
                gi = p_fh.tile([P, TQ], bf16, tag="f_gi", bufs=2)
                kcombine(gr, gi, mg)
                mu = [ps.tile([P, TQ], f32, tag="ps", name="ups%d" % q) for q in range(3)]
                for kt in range(FT):
                    w3 = (gtiles[kt][:, 3 * TQ:4 * TQ], gtiles[kt][:, 4 * TQ:5 * TQ],
                          gtiles[kt][:, 5 * TQ:6 * TQ])
                    kmm(mu, w3, kt, FT, h2_r[kt], h2_i[kt], h2_s[kt], jsl)
                ur = p_fh.tile([P, TQ], bf16, tag="f_ur", bufs=2)
                ui = p_fh.tile([P, TQ], bf16, tag="f_ui", bufs=2)
                kcombine(ur, ui, mu)
                sq1 = p_fh.tile([P, TQ], bf16, tag="f_sq1", bufs=2)
                STTv(out=sq1, in0=gr, scalar=1.0, in1=gr, op0=ALU.mult, op1=ALU.mult)
                STTv(out=qs_[j], in0=gi, scalar=1.0, in1=gi, op0=ALU.mult, op1=ALU.mult)
                TTp(out=qs_[j], in0=qs_[j], in1=sq1, op=ALU.add)
                t1 = p_fh.tile([P, TQ], bf16, tag="f_t1", bufs=2)
                t2 = p_fh.tile([P, TQ], bf16, tag="f_t2", bufs=2)
                TTv(out=t1, in0=gr, in1=ur, op=ALU.mult)
                TTp(out=t2, in0=gi, in1=ui, op=ALU.mult)
                TTv(out=hid_r[j], in0=t1, in1=t2, op=ALU.subtract)
                TTv(out=t1, in0=gr, in1=ui, op=ALU.mult)
                TTp(out=t2, in0=gi, in1=ur, op=ALU.mult)
                TTv(out=hid_i[j], in0=t1, in1=t2, op=ALU.add)
        if not NOFFN:
            # sentinel [P,1] zero tiles (bias operands) force each half-batch
            # of Sqrt ops to become ready only after the half's last qq is
            # written (qq adds are in-order on Pool, so qq[hi] done => all
            # done), and each Tanh half only after the half's last Sqrt. The
            # scheduler then runs each batch back-to-back on ACT: ~5 table
            # loads per iteration instead of one sqrt/exp pair per j tile.
            for half in range(2):
                lo, hi = half * 8, half * 8 + 7
                sent1 = p_sm.tile([P, 1], f32, name="sent1_%d" % half)
                TSv(out=sent1, in0=qs_[hi][:, 0:1], scalar1=0.0, scalar2=None,
                    op0=ALU.mult)
                for j in range(lo, hi + 1):   # batched Sqrt (one table window)
                    nc.scalar.activation(out=qs_[j], in_=qs_[j], func=AF.Sqrt,
                                         bias=sent1)
                sent2 = p_sm.tile([P, 1], f32, name="sent2_%d" % half)
                TSv(out=sent2, in0=qs_[hi][:, 0:1], scalar1=0.0, scalar2=None,
                    op0=ALU.mult)
                for j in range(lo, hi + 1):   # batched Tanh (exp/tanh set)
                    s_ = p_fh.tile([P, TQ], bf16, tag="f_s", bufs=16)
                    nc.scalar.activation(out=s_, in_=qs_[j], func=AF.Tanh, scale=0.5,
                                         bias=sent2)
                    TSv(out=s_, in0=s_, scalar1=0.5, scalar2=0.5, op0=ALU.mult,
                        op1=ALU.add)
                    TTv(out=hid_r[j], in0=hid_r[j], in1=s_, op=ALU.mult)
                    TTv(out=hid_i[j], in0=hid_i[j], in1=s_, op=ALU.mult)

        # ================= down proj (4-mult, kt-outer) + out =================
        es_ps.close()   # free the 6-bank pool; down needs 8 banks
        es_dn = ExitStack()
        ps_dn = es_dn.enter_context(tc.tile_pool(name="ps_dn", bufs=1, space="PSUM"))
        pd_r = [ps_dn.tile([P, TQ], f32, name='pdr%d' % ot) for ot in range(FT)]
        pd_i = [ps_dn.tile([P, TQ], f32, name='pdi%d' % ot) for ot in range(FT)]
        for kt in ([] if NOFFN else range(16)):
            w = p_fh.tile([P, 3 * D], bf16, tag="wdn", bufs=4, name="wdn", uniquify=True)
            if 'nowdma' not in ABLATE:
                nc.sync.dma_start(out=w, in_=io['wdn'][kt * P:(kt + 1) * P, :])
            st, sp = kt == 0, kt == 15
            for ot in range(FT):
                msl = slice(ot * P, (ot + 1) * P)
                nc.tensor.matmul(pd_r[ot], w[:, 0:D][:, msl], hid_r[kt], start=st, stop=False)
                nc.tensor.matmul(pd_r[ot], w[:, 2 * D:3 * D][:, msl], hid_i[kt],
                                 start=False, stop=sp)
                nc.tensor.matmul(pd_i[ot], w[:, D:2 * D][:, msl], hid_r[kt], start=st, stop=False)
                nc.tensor.matmul(pd_i[ot], w[:, 0:D][:, msl], hid_i[kt], start=False, stop=sp)
        for ot in range(FT):
            outr = p_keep.tile([P, TQ], f32, tag="ores", bufs=2)
            outi = p_keep.tile([P, TQ], f32, tag="ores", bufs=2)
            if NOFFN:
                TSv(out=outr, in0=res_r[ot], scalar1=1.0, scalar2=None, op0=ALU.mult)
                TSv(out=outi, in0=res_i[ot], scalar1=1.0, scalar2=None, op0=ALU.mult)
            else:
                TTv(out=outr, in0=pd_r[ot], in1=res_r[ot], op=ALU.add)
                TTv(out=outi, in0=pd_i[ot], in1=res_i[ot], op=ALU.add)
            nc.sync.dma_start(out=io['out_r'][ot * P:(ot + 1) * P, :], in_=outr)
            nc.sync.dma_start(out=io['out_i'][ot * P:(ot + 1) * P, :], in_=outi)
        es_dn.close()
        es_fh.close()
        es_keep.close()
        es_f.close()


def _build_module(n_iters=1, trivial_ln=False, trivial_b=True):
    import concourse.tile as tile
    from concourse import bacc, mybir

    f32 = mybir.dt.float32
    bf16 = mybir.dt.bfloat16
    nc = bacc.Bacc(None, target_bir_lowering=False, debug=False)
    with tile.TileContext(nc) as tc:
        with tc.tile_pool(name="dram", bufs=1, space="DRAM") as dram:
            io = {}

            def din(name, shape, dtype=f32):
                io[name] = dram.tile(shape, dtype, kind='ExternalInput', name=name,
                                     uniquify=False)

            din('xf_r', [D, S], bf16); din('xf_i', [D, S], bf16)
            din('wqkvo', [D, 4 * 3 * D], bf16)
            din('wgu', [D, 4 * 2 * 3 * TQ], bf16)
            din('wdn', [HID, 3 * D], bf16)
            din('cosk', [P, S], bf16); din('sink', [P, S], bf16)
            din('maskD', [P, 8 * P], bf16)
            din('shmat', [P, P], bf16)
            din('lnst', [1, 2 * 3 * TQ], bf16)
            if not trivial_ln:
                for ln in ['ln1', 'ln2']:
                    for q in ['gr', 'gi', 'br', 'bi']:
                        din(ln + '_' + q, [D])
            if not trivial_b:
                for nm in ['q', 'k', 'v', 'o']:
                    din('b_' + nm + '_r', [D]); din('b_' + nm + '_i', [D])
            io['out_r'] = dram.tile([D, TQ], f32, kind='ExternalOutput', name='out_r',
                                    uniquify=False)
            io['out_i'] = dram.tile([D, TQ], f32, kind='ExternalOutput', name='out_i',
                                    uniquify=False)

            if n_iters == 1:
                _emit_body(nc, tc, io, TRIV_LN=trivial_ln, TRIV_B=trivial_b)
            else:
                with tc.For_i(0, n_iters, 1):
                    _emit_body(nc, tc, io, TRIV_LN=trivial_ln, TRIV_B=trivial_b)
    nc.compile()
    return nc


def _prep_weights(full):
    """Host-side weight preprocessing -> packed bf16 arrays (shared by cores)."""
    import ml_dtypes
    bf = ml_dtypes.bfloat16

    def comps(lm, ph, karatsuba):
        mag = np.exp(lm.astype(np.float64))
        wr = (mag * np.cos(ph.astype(np.float64)))
        wi = (mag * np.sin(ph.astype(np.float64)))
        if karatsuba:
            return wr, wr + wi, wi - wr
        return wr, wi, -wi

    # qkvo: [D, 4, 3, D] (rows = in features, w.T layout)
    wqkvo = np.empty((D, 4, 3, D), dtype=bf)
    for m, nm in enumerate(['q', 'k', 'v', 'o']):
        kara = nm != 'v'
        c0, c1, c2 = comps(full[nm + '_lm'], full[nm + '_ph'], kara)
        wqkvo[:, m, 0, :] = c0.T.astype(bf)
        wqkvo[:, m, 1, :] = c1.T.astype(bf)
        wqkvo[:, m, 2, :] = c2.T.astype(bf)
    wqkvo = wqkvo.reshape(D, 4 * 3 * D)

    # gate/up: [D, G=4, 2, 3, 512]
    wgu = np.empty((D, 4, 2, 3, TQ), dtype=bf)
    for x, nm in enumerate(['gate', 'up']):
        c0, c1, c2 = comps(full[nm + '_lm'], full[nm + '_ph'], True)
        for g in range(4):
            osl = slice(g * TQ, (g + 1) * TQ)
            wgu[:, g, x, 0, :] = c0.T[:, osl].astype(bf)
            wgu[:, g, x, 1, :] = c1.T[:, osl].astype(bf)
            wgu[:, g, x, 2, :] = c2.T[:, osl].astype(bf)
    wgu = wgu.reshape(D, 4 * 2 * 3 * TQ)

    # down: [HID, 3, D]
    c0, c1, c2 = comps(full['down_lm'], full['down_ph'], False)
    wdn = np.empty((HID, 3, D), dtype=bf)
    wdn[:, 0, :] = c0.T.astype(bf)
    wdn[:, 1, :] = c1.T.astype(bf)
    wdn[:, 2, :] = c2.T.astype(bf)
    wdn = wdn.reshape(HID, 3 * D)

    # signed rotate-half permutation: out[m] = -pre[m+32] (m%64<32),
    #                                 out[m] = +pre[m-32] (m%64>=32)
    shmat = np.zeros((P, P), dtype=bf)
    for m in range(P):
        if m % HD < 32:
            shmat[m + 32, m] = -1.0
        else:
            shmat[m - 32, m] = 1.0
    return {'wqkvo': wqkvo, 'wgu': wgu, 'wdn': wdn, 'shmat': shmat}


def _host_inputs(x_real, x_imag, full, core, shared=None):
    import ml_dtypes
    bf = ml_dtypes.bfloat16
    if shared is None:
        shared = _prep_weights(full)
    b, hh = core // 2, core % 2
    own = np.arange(hh, S, 2)
    other = np.arange(1 - hh, S, 2)
    perm = np.concatenate([own, other])

    m = dict(shared)
    m['xf_r'] = np.ascontiguousarray(np.asarray(x_real)[b][perm].T.astype(bf))
    m['xf_i'] = np.ascontiguousarray(np.asarray(x_imag)[b][perm].T.astype(bf))

    xr64 = m['xf_r'].astype(np.float64)
    xi64 = m['xf_i'].astype(np.float64)
    lnst = np.empty((2, 3, TQ), dtype=np.float64)
    for ch in range(2):
        sl = slice(ch * TQ, (ch + 1) * TQ)
        mr = xr64[:, sl].mean(axis=0)
        mi = xi64[:, sl].mean(axis=0)
        var = (xr64[:, sl] ** 2 + xi64[:, sl] ** 2).mean(axis=0) - mr * mr - mi * mi
        iv = 1.0 / np.sqrt(var + EPS)
        lnst[ch, 0] = iv
        lnst[ch, 1] = mr * iv
        lnst[ch, 2] = mi * iv
    m['lnst'] = np.ascontiguousarray(lnst.reshape(1, 2 * 3 * TQ).astype(bf))

    invf = 1.0 / (10000.0 ** (np.arange(0, HD, 2, dtype=np.float64) / HD))
    pidx = (np.arange(P) % HD) % 32
    ang = perm[None, :].astype(np.float64) * invf[pidx][:, None]
    m['cosk'] = np.cos(ang).astype(bf)
    m['sink'] = np.sin(ang).astype(bf)

    # diagonal-block masks [P, 8*128]
    maskD = np.zeros((P, 8 * P), dtype=bf)
    for t in range(8):
        t4 = t % 4
        kg = 2 * (t4 * P + np.arange(P)) + (hh if t < 4 else 1 - hh)
        qg = 2 * (t4 * P + np.arange(P)) + hh
        maskD[:, t * P:(t + 1) * P] = (kg[:, None] <= qg[None, :]).astype(bf)
    m['maskD'] = maskD

    trivial_ln = _is_trivial_ln(full)
    if not trivial_ln:
        for ln in ['ln1', 'ln2']:
            for q in ['gr', 'gi', 'br', 'bi']:
                m[ln + '_' + q] = np.asarray(full[ln + '_' + q], dtype=np.float32)
    trivial_b = _is_trivial_b(full)
    if not trivial_b:
        for nm in ['q', 'k', 'v', 'o']:
            bm = np.asarray(full[nm + '_bm'], dtype=np.float64)
            bp = np.asarray(full[nm + '_bp'], dtype=np.float64)
            m['b_' + nm + '_r'] = (bm * np.cos(bp)).astype(np.float32)
            m['b_' + nm + '_i'] = (bm * np.sin(bp)).astype(np.float32)
    return m


def _is_trivial_ln(full):
    return (np.all(np.asarray(full['ln1_gr']) == 1) and np.all(np.asarray(full['ln2_gr']) == 1)
            and all(np.all(np.asarray(full[k]) == 0)
                    for k in ['ln1_gi', 'ln1_br', 'ln1_bi', 'ln2_gi', 'ln2_br', 'ln2_bi']))


def _is_trivial_b(full):
    return all(np.all(np.asarray(full[nm + '_bm']) == 0) for nm in ['q', 'k', 'v', 'o'])


def kernel(**inputs):
    from concourse.bass_utils import run_bass_kernel_spmd

    full = {k: np.asarray(v) for k, v in inputs.items()}
    x_real, x_imag = full['x_real'], full['x_imag']

    trivial_ln = _is_trivial_ln(full)
    trivial_b = _is_trivial_b(full)
    key = ('mod', trivial_ln, trivial_b)
    if key not in _CACHE:
        _CACHE[key] = _build_module(1, trivial_ln=trivial_ln, trivial_b=trivial_b)
    nc = _CACHE[key]

    shared = _prep_weights(full)
    in_maps = [_host_inputs(x_real, x_imag, full, c, shared) for c in range(NCORES)]
    res = run_bass_kernel_spmd(nc, in_maps, core_ids=list(range(NCORES)), trace=False)

    out = np.empty((2, B, S, D), dtype=np.float32)
    for c in range(NCORES):
        b, hh = c // 2, c % 2
        out[0, b, hh::2, :] = res.results[c]['out_r'].T
        out[1, b, hh::2, :] = res.results[c]['out_i'].T
    return out

